# revision 2
# baseline (speedup 1.0000x reference)
"""BlockRadiusMixer Trainium2 kernel.

Computes, for x [B, 4096] and Q [32, 128, 128]:
    z[b, n, :] = relu(x[b, n*128:(n+1)*128] @ Q[n])
    y = z.reshape(B, 4096);  y /= max(||y||_row, 1e-12)

Strategy: data-parallel over 8 NeuronCores (2048 batch rows each).
The contraction dim (d within a block) must live on SBUF partitions for
the tensor engine, so the host pre-transposes x: each core receives
xt = x_shard.T with shape [4096, 2048] (feature-major).  The kernel
streams batch-column tiles, runs the 32 block matmuls with Q_n as the
stationary operand, relu's PSUM->SBUF on the scalar engine, squares on
the scalar engine (bf16), reduces the squares across the 4096 features
with a ones-vector matmul accumulated in PSUM, broadcasts 1/norm back
across partitions with a K=1 matmul, scales on the vector engine, and
streams the (still transposed) result out.  The host transposes back.
"""

import numpy as np

import concourse.bass as bass
import concourse.tile as tile
from concourse import bacc, mybir
from concourse.bass_utils import run_bass_kernel_spmd

N_CORES = 8
BATCH = 16384
D = 4096
NBLK = 32
BD = 128
B_CORE = BATCH // N_CORES  # 2048
EPS2 = 1e-24  # eps**2 so that max(sqrt(s), eps) == sqrt(max(s, eps**2))

FP32 = mybir.dt.float32
BF16 = mybir.dt.bfloat16


def build_kernel(b_core: int = B_CORE, nb: int = 256, repeat: int = 1):
    """Builds + compiles the per-core Bass module. All 8 cores run the
    same NEFF on their own batch shard. repeat>1 wraps the whole pipeline
    in a hardware loop re-doing identical work — used only for timing
    (wall-clock slope between repeat=1 and repeat=K cancels transfer and
    dispatch overhead)."""
    nsteps = b_core // nb
    nc = bacc.Bacc(
        "TRN2",
        target_bir_lowering=False,
        debug=False,
        enable_asserts=False,
        num_devices=N_CORES,
    )
    xt = nc.dram_tensor("xt", [D, b_core], FP32, kind="ExternalInput").ap()
    q = nc.dram_tensor("q", [NBLK, BD, BD], FP32, kind="ExternalInput").ap()
    y = nc.dram_tensor("y", [D, b_core], FP32, kind="ExternalOutput").ap()

    with tile.TileContext(nc) as tc:
        with (
            tc.tile_pool(name="qpool", bufs=1) as qpool,
            tc.tile_pool(name="xpool", bufs=2) as xpool,
            tc.tile_pool(name="zpool", bufs=2) as zpool,
            tc.tile_pool(name="sqpool", bufs=3) as sqpool,
            tc.tile_pool(name="consts", bufs=1) as consts,
            tc.tile_pool(name="npool", bufs=2) as npool,
            tc.tile_pool(name="mm_psum", bufs=3, space="PSUM") as mm_psum,
            tc.tile_pool(name="s_psum", bufs=2, space="PSUM") as s_psum,
            tc.tile_pool(name="b_psum", bufs=2, space="PSUM") as b_psum,
        ):
            ones_col = consts.tile([BD, 1], BF16)  # lhsT of the sum-reduce matmul
            nc.vector.memset(ones_col[:], 1.0)
            ones_row = consts.tile([1, BD], FP32)  # lhsT of the broadcast matmul
            nc.vector.memset(ones_row[:], 1.0)

            xt_r = xt.rearrange("(n d) b -> d n b", d=BD)
            y_r = y.rearrange("(n d) b -> d n b", d=BD)

            def rep_body():
                # Q in SBUF: partition = d, free = (n, e)
                q_sb = qpool.tile([BD, NBLK, BD], FP32)
                nc.sync.dma_start(q_sb[:], q.rearrange("n d e -> d n e"))

                for t in range(nsteps):
                    bs = bass.ts(t, nb)
                    x_sb = xpool.tile([BD, NBLK, nb], FP32)
                    nc.sync.dma_start(x_sb[:], xt_r[:, :, bs])
                    z_sb = zpool.tile([BD, NBLK, nb], FP32)
                    s_ps = s_psum.tile([1, nb], FP32)

                    # block matmuls, two per PSUM bank so the relu reads 512-wide
                    for jp in range(NBLK // 2):
                        z_ps = mm_psum.tile([BD, 2, nb], FP32)
                        for h in range(2):
                            n = 2 * jp + h
                            nc.tensor.matmul(
                                z_ps[:, h, :],
                                q_sb[:, n, :],
                                x_sb[:, n, :],
                                start=True,
                                stop=True,
                            )
                        nc.scalar.activation(
                            z_sb[:, 2 * jp : 2 * jp + 2, :],
                            z_ps[:],
                            mybir.ActivationFunctionType.Relu,
                        )

                    # squared relu (bf16) -> ones-matmul accumulates sum over
                    # all 4096 features into s_ps[1, nb]
                    for jc in range(NBLK // 4):
                        sq = sqpool.tile([BD, 4, nb], BF16)
                        nc.scalar.activation(
                            sq[:],
                            z_sb[:, 4 * jc : 4 * jc + 4, :],
                            mybir.ActivationFunctionType.Square,
                        )
                        for h in range(4):
                            n = 4 * jc + h
                            nc.tensor.matmul(
                                s_ps[:],
                                ones_col[:],
                                sq[:, h, :],
                                start=(n == 0),
                                stop=(n == NBLK - 1),
                            )

                    # recip = 1 / sqrt(max(s, eps^2))
                    nrm = npool.tile([1, nb], FP32)
                    nc.vector.tensor_scalar_max(nrm[:], s_ps[:], EPS2)
                    nc.scalar.sqrt(nrm[:], nrm[:])
                    recip = npool.tile([1, nb], FP32)
                    nc.vector.reciprocal(recip[:], nrm[:])

                    # broadcast recip across the 128 partitions via a K=1 matmul
                    bc_ps = b_psum.tile([BD, nb], FP32)
                    nc.tensor.matmul(
                        bc_ps[:], ones_row[:], recip[:], start=True, stop=True
                    )

                    # y = relu(z) * recip  (in place), then store
                    nc.vector.tensor_mul(
                        z_sb[:],
                        z_sb[:],
                        bc_ps[:, None, :].broadcast_to([BD, NBLK, nb]),
                    )
                    nc.sync.dma_start(y_r[:, :, bs], z_sb[:])

            if repeat == 1:
                rep_body()
            else:
                with tc.For_i(0, repeat, 1, hint_engines=(mybir.EngineType.PE,)):
                    rep_body()

    nc.compile()
    return nc


_NC_CACHE: dict = {}


def _get_nc():
    if "nc" not in _NC_CACHE:
        _NC_CACHE["nc"] = build_kernel()
    return _NC_CACHE["nc"]


def shard_inputs(x: np.ndarray, Q: np.ndarray) -> list[dict]:
    """Per-core input maps: xt = per-core batch shard, transposed."""
    x = np.asarray(x, dtype=np.float32)
    Q = np.ascontiguousarray(np.asarray(Q, dtype=np.float32))
    xs = x.reshape(N_CORES, B_CORE, D)
    return [
        {"xt": np.ascontiguousarray(xs[c].T), "q": Q} for c in range(N_CORES)
    ]


def unshard_output(results: list[dict]) -> np.ndarray:
    out = np.empty((BATCH, D), dtype=np.float32)
    for c in range(N_CORES):
        out[c * B_CORE : (c + 1) * B_CORE] = results[c]["y"].T
    return out


def kernel(x, Q) -> np.ndarray:
    nc = _get_nc()
    in_maps = shard_inputs(x, Q)
    res = run_bass_kernel_spmd(nc, in_maps, core_ids=list(range(N_CORES)))
    return unshard_output(res.results)


# revision 4
# speedup vs baseline: 1.0570x; 1.0570x over previous
"""BlockRadiusMixer Trainium2 kernel.

Computes, for x [B, 4096] and Q [32, 128, 128]:
    z[b, n, :] = relu(x[b, n*128:(n+1)*128] @ Q[n])
    y = z.reshape(B, 4096);  y /= max(||y||_row, 1e-12)

Strategy: data-parallel over 8 NeuronCores (2048 batch rows each).
The contraction dim (d within a block) must live on SBUF partitions for
the tensor engine, so the host pre-tiles x into the exact per-DMA-step
SBUF image xtt[t, d, n, b] = x_shard[t*NB + b, n*128 + d]: every DMA is
then a fully contiguous block (32KB per partition).  The kernel runs the
32 block matmuls with Q_n stationary, relu's PSUM->SBUF on the scalar
engine, squares (bf16) on the scalar engine, reduces the squares across
all 4096 features with a ones-vector matmul accumulated in PSUM,
broadcasts 1/norm across partitions with a K=1 matmul, scales on the
vector engine, and streams the result out in the same tiled layout,
which the host un-tiles.
"""

import numpy as np

import concourse.bass as bass
import concourse.tile as tile
from concourse import bacc, mybir
from concourse.bass_utils import run_bass_kernel_spmd

N_CORES = 8
BATCH = 16384
D = 4096
NBLK = 32
BD = 128
B_CORE = BATCH // N_CORES  # 2048
NB = 256  # batch columns per pipeline step
NSTEPS = B_CORE // NB
EPS2 = 1e-24  # eps**2 so that max(sqrt(s), eps) == sqrt(max(s, eps**2))

FP32 = mybir.dt.float32
BF16 = mybir.dt.bfloat16


def build_kernel(nsteps: int = NSTEPS, nb: int = NB, repeat: int = 1):
    """Builds + compiles the per-core Bass module. All 8 cores run the
    same NEFF on their own batch shard. repeat>1 wraps the whole pipeline
    in a hardware loop re-doing identical work — used only for timing
    (wall-clock slope between repeat=1 and repeat=K cancels transfer and
    dispatch overhead)."""
    nc = bacc.Bacc(
        "TRN2",
        target_bir_lowering=False,
        debug=False,
        enable_asserts=False,
        num_devices=N_CORES,
    )
    xt = nc.dram_tensor("xt", [nsteps, BD, NBLK * nb], FP32, kind="ExternalInput").ap()
    q = nc.dram_tensor("q", [BD, NBLK * BD], FP32, kind="ExternalInput").ap()
    y = nc.dram_tensor("y", [nsteps, BD, NBLK * nb], FP32, kind="ExternalOutput").ap()

    with tile.TileContext(nc) as tc:
        with (
            tc.tile_pool(name="qpool", bufs=1) as qpool,
            tc.tile_pool(name="xpool", bufs=3) as xpool,
            tc.tile_pool(name="zpool", bufs=2) as zpool,
            tc.tile_pool(name="sqpool", bufs=3) as sqpool,
            tc.tile_pool(name="consts", bufs=1) as consts,
            tc.tile_pool(name="npool", bufs=2) as npool,
            tc.tile_pool(name="mm_psum", bufs=3, space="PSUM") as mm_psum,
            tc.tile_pool(name="s_psum", bufs=2, space="PSUM") as s_psum,
            tc.tile_pool(name="b_psum", bufs=2, space="PSUM") as b_psum,
        ):
            ones_col = consts.tile([BD, 1], BF16)  # lhsT of the sum-reduce matmul
            nc.vector.memset(ones_col[:], 1.0)
            ones_row = consts.tile([1, BD], FP32)  # lhsT of the broadcast matmul
            nc.vector.memset(ones_row[:], 1.0)

            def rep_body():
                # Q in SBUF: partition = d, free = (n, e)
                q_sb = qpool.tile([BD, NBLK, BD], FP32)
                nc.sync.dma_start(q_sb[:], q.rearrange("d (n e) -> d n e", e=BD))

                for t in range(nsteps):
                    x_sb = xpool.tile([BD, NBLK, nb], FP32)
                    nc.sync.dma_start(
                        x_sb[:], xt[t].rearrange("d (n b) -> d n b", b=nb)
                    )
                    z_sb = zpool.tile([BD, NBLK, nb], FP32)
                    s_ps = s_psum.tile([1, nb], FP32)

                    # block matmuls, two per PSUM bank so the relu reads 512-wide
                    for jp in range(NBLK // 2):
                        z_ps = mm_psum.tile([BD, 2, nb], FP32)
                        for h in range(2):
                            n = 2 * jp + h
                            nc.tensor.matmul(
                                z_ps[:, h, :],
                                q_sb[:, n, :],
                                x_sb[:, n, :],
                                start=True,
                                stop=True,
                            )
                        nc.scalar.activation(
                            z_sb[:, 2 * jp : 2 * jp + 2, :],
                            z_ps[:],
                            mybir.ActivationFunctionType.Relu,
                        )

                    # squared relu (bf16) -> ones-matmul accumulates sum over
                    # all 4096 features into s_ps[1, nb]
                    for jc in range(NBLK // 4):
                        sq = sqpool.tile([BD, 4, nb], BF16)
                        nc.scalar.activation(
                            sq[:],
                            z_sb[:, 4 * jc : 4 * jc + 4, :],
                            mybir.ActivationFunctionType.Square,
                        )
                        for h in range(4):
                            n = 4 * jc + h
                            nc.tensor.matmul(
                                s_ps[:],
                                ones_col[:],
                                sq[:, h, :],
                                start=(n == 0),
                                stop=(n == NBLK - 1),
                            )

                    # recip = 1 / sqrt(max(s, eps^2))
                    nrm = npool.tile([1, nb], FP32)
                    nc.vector.tensor_scalar_max(nrm[:], s_ps[:], EPS2)
                    nc.scalar.sqrt(nrm[:], nrm[:])
                    recip = npool.tile([1, nb], FP32)
                    nc.vector.reciprocal(recip[:], nrm[:])

                    # broadcast recip across the 128 partitions via a K=1 matmul
                    bc_ps = b_psum.tile([BD, nb], FP32)
                    nc.tensor.matmul(
                        bc_ps[:], ones_row[:], recip[:], start=True, stop=True
                    )

                    # y = relu(z) * recip  (in place), then store
                    nc.vector.tensor_mul(
                        z_sb[:],
                        z_sb[:],
                        bc_ps[:, None, :].broadcast_to([BD, NBLK, nb]),
                    )
                    nc.sync.dma_start(
                        y[t].rearrange("d (n b) -> d n b", b=nb), z_sb[:]
                    )

            if repeat == 1:
                rep_body()
            else:
                with tc.For_i(0, repeat, 1, hint_engines=(mybir.EngineType.PE,)):
                    rep_body()

    nc.compile()
    return nc


_NC_CACHE: dict = {}


def _get_nc():
    if "nc" not in _NC_CACHE:
        _NC_CACHE["nc"] = build_kernel()
    return _NC_CACHE["nc"]


def shard_inputs(x: np.ndarray, Q: np.ndarray) -> list[dict]:
    """Per-core input maps in the pre-tiled DMA-friendly layout:
    xtt[t, d, n*nb + b] = x_shard[t*NB + b, n*128 + d]."""
    x = np.asarray(x, dtype=np.float32)
    Q = np.asarray(Q, dtype=np.float32)
    qh = np.ascontiguousarray(Q.transpose(1, 0, 2)).reshape(BD, NBLK * BD)
    xs = x.reshape(N_CORES, NSTEPS, NB, NBLK, BD)  # [c, t, b, n, d]
    in_maps = []
    for c in range(N_CORES):
        xtt = np.ascontiguousarray(xs[c].transpose(0, 3, 2, 1))  # [t, d, n, b]
        in_maps.append(
            {"xt": xtt.reshape(NSTEPS, BD, NBLK * NB), "q": qh}
        )
    return in_maps


def unshard_output(results: list[dict]) -> np.ndarray:
    out = np.empty((N_CORES, NSTEPS, NB, NBLK, BD), dtype=np.float32)
    for c in range(N_CORES):
        ytt = results[c]["y"].reshape(NSTEPS, BD, NBLK, NB)
        out[c] = ytt.transpose(0, 3, 2, 1)  # -> [t, b, n, d]
    return out.reshape(BATCH, D)


def kernel(x, Q) -> np.ndarray:
    nc = _get_nc()
    in_maps = shard_inputs(x, Q)
    res = run_bass_kernel_spmd(nc, in_maps, core_ids=list(range(N_CORES)))
    return unshard_output(res.results)
